# revision 9
# baseline (speedup 1.0000x reference)
"""Trainium2 Bass kernel for a 6-layer transformer decoder (nn_Decoder).

Sharding: pure data-parallel over batch — core b computes batch element b.
No collectives. Matmuls in bf16 (weights host-pretransposed + cast), softmax /
layernorm / residual in f32.

Outputs (x, sa_ws, ca_ws) exactly mirror the reference:
  x      [B, LD, D]          final hidden states
  sa_ws  [NL, B, H, LD, LD]  self-attention weights (causal upper blocks are
                             never written on device; host buffer is zeros)
  ca_ws  [NL, B, H, LD, LE]  cross-attention weights
"""

import contextlib
import os
import sys

for _p in ("/opt/trn_rl_repo", "/root/.axon_site/_ro/trn_rl_repo"):
    if os.path.isdir(_p) and _p not in sys.path:
        sys.path.append(_p)

import numpy as np
import ml_dtypes

import concourse.bass as bass
import concourse.mybir as mybir
import concourse.tile as tile
from concourse import bacc
from concourse import bass_utils

BF16 = ml_dtypes.bfloat16
B, LD, LE = 8, 512, 512
V, D, H, DK, FF = 32000, 512, 8, 64, 2048
NL = 6
SCALE = float(1.0 / np.sqrt(np.float32(DK)).astype(np.float32))
NEG = -1.0e9
F32 = mybir.dt.float32
BF = mybir.dt.bfloat16

N_CORES = int(os.environ.get("KERNEL_N_CORES", "8"))
NL_BUILD = int(os.environ.get("KERNEL_NL", str(NL)))


def sin_table(n_pos, dim):
    pos = np.arange(n_pos)[:, None].astype(np.float64)
    j = np.arange(dim)[None, :]
    angle = pos / np.power(10000.0, 2 * (j // 2) / dim)
    t = np.zeros((n_pos, dim))
    t[:, 0::2] = np.sin(angle[:, 0::2])
    t[:, 1::2] = np.cos(angle[:, 1::2])
    return t.astype(np.float32)


# --------------------------------------------------------------------------
# device kernel builder
# --------------------------------------------------------------------------

_BUILD_CACHE = {}
LAST_RESULT = None


def build_kernel(affine: bool, nl: int = NL_BUILD, n_cores: int = N_CORES):
    key = (affine, nl, n_cores)
    if key in _BUILD_CACHE:
        return _BUILD_CACHE[key]

    nc = bacc.Bacc("TRN2", target_bir_lowering=False, debug=False,
                   num_devices=n_cores)

    def din(name, shape, dt):
        return nc.dram_tensor(name, shape, dt, kind="ExternalInput").ap()

    def dout(name, shape, dt):
        return nc.dram_tensor(name, shape, dt, kind="ExternalOutput").ap()

    x0 = din("x0", [LD, D], F32)
    x0T = din("x0T", [D, LD], BF)
    encT = din("encT", [D, LE], BF)
    wqt = {p: din(f"{p}_wqt", [nl, D, D], BF) for p in ("sa", "ca")}
    wkt = {p: din(f"{p}_wkt", [nl, D, D], BF) for p in ("sa", "ca")}
    wvt = {p: din(f"{p}_wvt", [nl, D, D], BF) for p in ("sa", "ca")}
    wot = {p: din(f"{p}_wot", [nl, D, D], BF) for p in ("sa", "ca")}
    bqs = {p: din(f"{p}_bqs", [nl, D], F32) for p in ("sa", "ca")}
    bk = {p: din(f"{p}_bk", [nl, D], F32) for p in ("sa", "ca")}
    bv = {p: din(f"{p}_bv", [nl, D], F32) for p in ("sa", "ca")}
    bo = {p: din(f"{p}_bo", [nl, D], BF) for p in ("sa", "ca")}
    w1t = din("w1t", [nl, D, FF], BF)
    w2t = din("w2t", [nl, FF, D], BF)
    b1 = din("b1", [nl, FF], F32)
    b2r = din("b2r", [nl, D], BF)
    # additive score bias for self-attn [q,k] orientation: causal tri at the
    # diagonal block + dec-pad columns (one [128,512] tile per q-chunk)
    bias_qk = din("bias_qk", [4, 128, 512], F32)
    triT = din("triT", [128, 128], F32)     # transposed tri for [k,q] blocks
    pad_dec = din("pad_dec", [LD], F32)     # -1e9 at dec pad positions
    pad_enc_row = din("pad_enc_row", [1, LE], BF)
    pad_enc = din("pad_enc", [LE], F32)
    ident_f_d = din("ident_f", [128, 128], F32)
    ident_b_d = din("ident_b", [128, 128], BF)
    ones1_d = din("ones1", [1, 128], BF)
    if affine:
        ln_g = {p: din(f"{p}_g", [nl, D], F32) for p in ("sa", "ca", "ffn")}
        ln_b = {p: din(f"{p}_b", [nl, D], F32) for p in ("sa", "ca", "ffn")}

    x_out = dout("x_out", [LD, D], F32)
    saw = dout("saw", [nl, H, LD, LD], F32)
    caw = dout("caw", [nl, H, LD, LE], F32)

    with tile.TileContext(nc) as tc:
        with contextlib.ExitStack() as ctx:
            consts = ctx.enter_context(tc.tile_pool(name="consts", bufs=1))
            pw = ctx.enter_context(tc.tile_pool(name="pw", bufs=1))
            pact = ctx.enter_context(tc.tile_pool(name="pact", bufs=2))
            pes = ctx.enter_context(tc.tile_pool(name="pes", bufs=2))
            psm = ctx.enter_context(tc.tile_pool(name="psm", bufs=2))
            pps = ctx.enter_context(tc.tile_pool(name="pps", bufs=3, space="PSUM"))
            ptr = ctx.enter_context(tc.tile_pool(name="ptr", bufs=2, space="PSUM"))
            psml = ctx.enter_context(tc.tile_pool(name="psml", bufs=2, space="PSUM"))

            # ---- constants ----
            ident_f = consts.tile([128, 128], F32, tag="ident_f")
            nc.sync.dma_start(out=ident_f, in_=ident_f_d)
            ident_b = consts.tile([128, 128], BF, tag="ident_b")
            nc.sync.dma_start(out=ident_b, in_=ident_b_d)
            ones1 = consts.tile([1, 128], BF, tag="ones1")
            nc.sync.dma_start(out=ones1, in_=ones1_d)
            eps_t = consts.tile([128, 1], F32, tag="eps_t")
            nc.vector.memset(eps_t, 1e-5)
            triT_sb = consts.tile([128, 128], F32, tag="triT_sb")
            nc.sync.dma_start(out=triT_sb, in_=triT)
            bias_qk_sb = []
            for qc in range(4):
                t = consts.tile([128, 512], F32, tag=f"bias_qk_{qc}")
                nc.sync.dma_start(out=t, in_=bias_qk[qc])
                bias_qk_sb.append(t)
            pad_dec_sb = consts.tile([128, 4], F32, tag="pad_dec_sb")
            nc.sync.dma_start(out=pad_dec_sb, in_=pad_dec.rearrange("(c p) -> p c", p=128))
            pad_enc_sb = consts.tile([128, 4], F32, tag="pad_enc_sb")
            nc.sync.dma_start(out=pad_enc_sb, in_=pad_enc.rearrange("(c p) -> p c", p=128))
            pad_enc_row_sb = consts.tile([1, 512], BF, tag="pad_enc_row_sb")
            nc.sync.dma_start(out=pad_enc_row_sb, in_=pad_enc_row)

            # ---- encoder (layer-invariant) ----
            encT_sb = []
            for kt in range(4):
                t = consts.tile([128, 512], BF, tag=f"encT_{kt}")
                nc.sync.dma_start(out=t, in_=encT[kt * 128:(kt + 1) * 128, :])
                encT_sb.append(t)

            debug = os.environ.get("KERNEL_DEBUG") == "1"

            # ---- residual stream ----
            h = []
            hT = []
            for tt in range(4):
                t = pact.tile([128, 512], F32, tag=f"h_{tt}")
                nc.sync.dma_start(out=t, in_=x0[tt * 128:(tt + 1) * 128, :])
                h.append(t)
            for cc in range(4):
                t = pact.tile([128, 512], BF, tag=f"hT_{cc}")
                nc.sync.dma_start(out=t, in_=x0T[cc * 128:(cc + 1) * 128, :])
                hT.append(t)

            def load_w4(name, dram, l):
                out = []
                for kt in range(4):
                    t = pw.tile([128, 512], BF, tag=f"{name}_{kt}")
                    nc.sync.dma_start(out=t, in_=dram[l, kt * 128:(kt + 1) * 128, :])
                    out.append(t)
                return out

            def load_bias_col(name, dram, l, cols):
                t = pw.tile([128, cols], F32, tag=name)
                nc.sync.dma_start(out=t, in_=dram[l].rearrange("(c p) -> p c", p=128))
                return t

            def load_row(name, dram, l):
                t = pw.tile([1, 512], BF, tag=name)
                nc.sync.dma_start(out=t, in_=dram[l:l + 1, :])
                return t

            def residual_ln(tt, psum_o, gb_sb):
                """r = psum_o + h[tt]; h[tt] <- LN(r); returns bf16 copy."""
                r = psm.tile([128, 512], F32, tag="r")
                nc.vector.tensor_add(r, psum_o, h[tt])
                stats = psm.tile([128, 6], F32, tag="stats")
                nc.vector.bn_stats(out=stats, in_=r)
                mv = psm.tile([128, 2], F32, tag="mv")
                nc.vector.bn_aggr(out=mv, in_=stats)
                sd = psm.tile([128, 1], F32, tag="sd")
                nc.scalar.activation(out=sd, in_=mv[:, 1:2],
                                     func=mybir.ActivationFunctionType.Sqrt,
                                     bias=eps_t)
                rstd = psm.tile([128, 1], F32, tag="rstd")
                nc.vector.reciprocal(out=rstd, in_=sd)
                hn = pact.tile([128, 512], F32, tag=f"h_{tt}")
                hb = psm.tile([128, 512], BF, tag=f"hb_{tt}")
                if affine:
                    gt, bt = gb_sb
                    tmp = psm.tile([128, 512], F32, tag="ln_tmp")
                    nc.vector.tensor_scalar(out=tmp, in0=r, scalar1=mv[:, 0:1],
                                            scalar2=rstd, op0=mybir.AluOpType.subtract,
                                            op1=mybir.AluOpType.mult)
                    nc.vector.tensor_mul(tmp, tmp, gt)
                    nc.vector.tensor_add(hn, tmp, bt)
                    nc.vector.tensor_copy(out=hb, in_=hn)
                else:
                    nc.vector.tensor_scalar(out=hn, in0=r, scalar1=mv[:, 0:1],
                                            scalar2=rstd, op0=mybir.AluOpType.subtract,
                                            op1=mybir.AluOpType.mult)
                    nc.vector.tensor_scalar(out=hb, in0=r, scalar1=mv[:, 0:1],
                                            scalar2=rstd, op0=mybir.AluOpType.subtract,
                                            op1=mybir.AluOpType.mult)
                h[tt] = hn
                return hb

            def retranspose(hbs):
                """hbs: 4 bf16 [128(tok),512(ch)] tiles -> update hT."""
                for cc in range(4):
                    pt = ptr.tile([128, 512], BF, tag="psum_hT")
                    for tt in range(4):
                        nc.tensor.transpose(pt[:, tt * 128:(tt + 1) * 128],
                                            hbs[tt][:, cc * 128:(cc + 1) * 128], ident_b)
                    t = pact.tile([128, 512], BF, tag=f"hT_{cc}")
                    nc.scalar.activation(out=t, in_=pt,
                                         func=mybir.ActivationFunctionType.Copy)
                    hT[cc] = t

            def gb_tiles(pre, l):
                if not affine:
                    return None
                grow = pw.tile([1, 512], F32, tag=f"g_{pre}")
                nc.sync.dma_start(out=grow, in_=ln_g[pre][l:l + 1, :])
                brow = pw.tile([1, 512], F32, tag=f"b_{pre}")
                nc.sync.dma_start(out=brow, in_=ln_b[pre][l:l + 1, :])
                gt = pw.tile([128, 512], F32, tag=f"gt_{pre}")
                nc.gpsimd.partition_broadcast(gt, grow, channels=128)
                bt = pw.tile([128, 512], F32, tag=f"bt_{pre}")
                nc.gpsimd.partition_broadcast(bt, brow, channels=128)
                return (gt, bt)

            def attention(pre, l, causal, kvT_src, out_dram):
                """One attention sublayer. kvT_src: 4 [128ch,512tok] bf16 tiles
                (hT for self-attn, encT_sb for cross-attn)."""
                wq_sb = load_w4(f"{pre}_wq", wqt[pre], l)
                wk_sb = load_w4(f"{pre}_wk", wkt[pre], l)
                wv_sb = load_w4(f"{pre}_wv", wvt[pre], l)
                wo_sb = load_w4(f"{pre}_wo", wot[pre], l)
                bqs_sb = load_bias_col(f"{pre}_bqs_sb", bqs[pre], l, 4)
                bk_sb = load_bias_col(f"{pre}_bk_sb", bk[pre], l, 4)
                bv_sb = load_bias_col(f"{pre}_bv_sb", bv[pre], l, 4)
                bo_sb = load_row(f"{pre}_bo_sb", bo[pre], l)

                # qT[e,tok] (pre-scaled), kT[e,tok], v[tok,e]
                qT = []
                kT = []
                v = []
                for ec in range(4):
                    pq = pps.tile([128, 512], F32, tag="mm")
                    for kt in range(4):
                        nc.tensor.matmul(pq, wq_sb[kt][:, ec * 128:(ec + 1) * 128],
                                         hT[kt], start=kt == 0, stop=kt == 3)
                    qt_t = pes.tile([128, 512], BF, tag=f"qT_{ec}", bufs=1)
                    nc.scalar.activation(out=qt_t, in_=pq,
                                         func=mybir.ActivationFunctionType.Identity,
                                         bias=bqs_sb[:, ec:ec + 1], scale=SCALE)
                    qT.append(qt_t)
                for ec in range(4):
                    pk = pps.tile([128, 512], F32, tag="mm")
                    for kt in range(4):
                        nc.tensor.matmul(pk, wk_sb[kt][:, ec * 128:(ec + 1) * 128],
                                         kvT_src[kt], start=kt == 0, stop=kt == 3)
                    kt_t = pes.tile([128, 512], BF, tag=f"kT_{ec}", bufs=1)
                    nc.scalar.activation(out=kt_t, in_=pk,
                                         func=mybir.ActivationFunctionType.Identity,
                                         bias=bk_sb[:, ec:ec + 1])
                    kT.append(kt_t)
                for tt in range(4):
                    pv = pps.tile([128, 512], F32, tag="mm")
                    for kt in range(4):
                        nc.tensor.matmul(pv, kvT_src[kt][:, tt * 128:(tt + 1) * 128],
                                         wv_sb[kt], start=kt == 0, stop=kt == 3)
                    v_t = pes.tile([128, 512], BF, tag=f"v_{tt}")
                    nc.scalar.activation(out=v_t, in_=pv,
                                         func=mybir.ActivationFunctionType.Copy)
                    v.append(v_t)

                ctxT = []
                for hp in range(4):
                    hA, hB = 2 * hp, 2 * hp + 1
                    qTp, kTp = qT[hp], kT[hp]
                    # ---- scores [q,k] + softmax stats + w output ----
                    raccs = []
                    for qc in range(4):
                        FD = 128 * (qc + 1) if causal else 512
                        acc = psm.tile([128, 2], F32, tag="acc")
                        exps = []
                        for ab in range(2):
                            ps = pps.tile([128, 512], F32, tag="mm")
                            lo = 64 * ab
                            nc.tensor.matmul(ps[:, 0:FD],
                                             qTp[lo:lo + 64, qc * 128:(qc + 1) * 128],
                                             kTp[lo:lo + 64, 0:FD],
                                             start=True, stop=causal,
                                             tile_position=(lo, 0))
                            if not causal:
                                nc.tensor.matmul(ps[:, 0:FD], ones1,
                                                 pad_enc_row_sb[0:1, 0:FD],
                                                 start=False, stop=True)
                            if causal:
                                nc.vector.tensor_tensor(
                                    out=ps[:, 0:FD], in0=ps[:, 0:FD],
                                    in1=bias_qk_sb[qc][:, 0:FD],
                                    op=mybir.AluOpType.add)
                            es = psm.tile([128, 512], F32, tag=f"exps_{ab}")
                            nc.scalar.activation(out=es[:, 0:FD], in_=ps[:, 0:FD],
                                                 func=mybir.ActivationFunctionType.Exp,
                                                 accum_out=acc[:, ab:ab + 1])
                            exps.append(es)
                        acc2 = psm.tile([128, 2], F32, tag="acc2")
                        nc.vector.tensor_scalar_add(acc2, acc, 1e-30)
                        racc = psm.tile([128, 2], F32, tag="racc", bufs=4)
                        nc.vector.reciprocal(out=racc, in_=acc2)
                        raccs.append(racc)
                        for ab in range(2):
                            ws = psm.tile([128, 512], F32, tag=f"ws_{ab}", bufs=3)
                            nc.vector.tensor_scalar(out=ws[:, 0:FD], in0=exps[ab][:, 0:FD],
                                                    scalar1=racc[:, ab:ab + 1],
                                                    scalar2=None,
                                                    op0=mybir.AluOpType.mult)
                            head = hA if ab == 0 else hB
                            nc.sync.dma_start(
                                out=out_dram[l, head, qc * 128:(qc + 1) * 128, 0:FD],
                                in_=ws[:, 0:FD])
    # ---- recip in transposed orientation -> bcast tiles ----
                    # transpose outputs must land at PSUM partition 0:
                    # one PSUM tile per head row
                    prT_a = psml.tile([1, 512], F32, tag="prT_a", bufs=1)
                    prT_b = psml.tile([1, 512], F32, tag="prT_b", bufs=1)
                    for qc in range(4):
                        nc.tensor.transpose(prT_a[:, qc * 128:(qc + 1) * 128],
                                            raccs[qc][:, 0:1], ident_f)
                        nc.tensor.transpose(prT_b[:, qc * 128:(qc + 1) * 128],
                                            raccs[qc][:, 1:2], ident_f)
                    rT_a = psm.tile([1, 512], F32, tag="rT_a")
                    rT_b = psm.tile([1, 512], F32, tag="rT_b")
                    nc.scalar.activation(out=rT_a, in_=prT_a,
                                         func=mybir.ActivationFunctionType.Copy)
                    nc.scalar.activation(out=rT_b, in_=prT_b,
                                         func=mybir.ActivationFunctionType.Copy)
                    rbc_a = psm.tile([64, 512], F32, tag="rbc_a")
                    rbc_b = psm.tile([64, 512], F32, tag="rbc_b")
                    nc.gpsimd.partition_broadcast(rbc_a, rT_a, channels=64)
                    nc.gpsimd.partition_broadcast(rbc_b, rT_b, channels=64)

                    # ---- scores^T [k,q] + exp (pad as per-partition bias) ----
                    expsT = {0: [], 1: []}
                    for kc in range(4):
                        qlo = kc * 128 if causal else 0
                        FDq = 512 - qlo
                        pad_col = pad_dec_sb if causal else pad_enc_sb
                        for ab in range(2):
                            pst = pps.tile([128, 512], F32, tag="mm")
                            lo = 64 * ab
                            nc.tensor.matmul(pst[:, 0:FDq],
                                             kTp[lo:lo + 64, kc * 128:(kc + 1) * 128],
                                             qTp[lo:lo + 64, qlo:512],
                                             start=True, stop=True,
                                             tile_position=(lo, 0))
                            if causal:
                                nc.vector.tensor_tensor(
                                    out=pst[:, 0:128], in0=pst[:, 0:128],
                                    in1=triT_sb, op=mybir.AluOpType.add)
                            est = pes.tile([128, 512], BF, tag=f"expsT_{ab}_{kc}", bufs=1)
                            nc.scalar.activation(out=est[:, 0:FDq], in_=pst[:, 0:FDq],
                                                 func=mybir.ActivationFunctionType.Exp,
                                                 bias=pad_col[:, kc:kc + 1])
                            expsT[ab].append(est)
                    # ---- ctx^T[d, q] ----
                    pctx = pps.tile([128, 512], F32, tag="mm")
                    for kc in range(4):
                        qlo = kc * 128 if causal else 0
                        FDq = 512 - qlo
                        nc.tensor.matmul(pctx[0:64, qlo:512],
                                         v[kc][:, hA * 64:hA * 64 + 64],
                                         expsT[0][kc][:, 0:FDq],
                                         start=kc == 0, stop=kc == 3,
                                         tile_position=(0, 0))
                        nc.tensor.matmul(pctx[64:128, qlo:512],
                                         v[kc][:, hB * 64:hB * 64 + 64],
                                         expsT[1][kc][:, 0:FDq],
                                         start=kc == 0, stop=kc == 3,
                                         tile_position=(0, 64))
                    ct = pes.tile([128, 512], BF, tag=f"ctxT_{hp}")
                    nc.vector.tensor_mul(ct[0:64, :], pctx[0:64, :], rbc_a)
                    nc.vector.tensor_mul(ct[64:128, :], pctx[64:128, :], rbc_b)
                    # v bias: ctx += bv (sum of attention weights = 1)
                    nc.vector.tensor_scalar_add(ct[0:64, :], ct[0:64, :],
                                                bv_sb[0:64, hp:hp + 1])
                    nc.vector.tensor_scalar_add(ct[64:128, :], ct[64:128, :],
                                                bv_sb[64:128, hp:hp + 1])
                    ctxT.append(ct)
                    if debug and l == 0 and pre == "sa" and hp == 0:
                        nc.sync.dma_start(out=dbg_rbc[0:64, :], in_=rbc_a)
                        nc.sync.dma_start(out=dbg_rbc[64:128, :], in_=rbc_b)

                if debug and l == 0 and pre == "sa":
                    for dc in range(4):
                        nc.sync.dma_start(out=dbg_ctxT[dc * 128:(dc + 1) * 128, :],
                                          in_=ctxT[dc])

                # ---- output projection + residual + LN ----
                gb = gb_tiles(pre, l)
                hbs = []
                for tt in range(4):
                    po = pps.tile([128, 512], F32, tag="mm")
                    for dc in range(4):
                        nc.tensor.matmul(po, ctxT[dc][:, tt * 128:(tt + 1) * 128],
                                         wo_sb[dc], start=dc == 0, stop=False)
                    nc.tensor.matmul(po, ones1, bo_sb, start=False, stop=True)
                    hbs.append(residual_ln(tt, po, gb))
                retranspose(hbs)

            def ffn(l):
                w1_sb = []
                for kt in range(4):
                    t = pw.tile([128, 2048], BF, tag=f"w1_{kt}")
                    nc.sync.dma_start(out=t, in_=w1t[l, kt * 128:(kt + 1) * 128, :])
                    w1_sb.append(t)
                w2_sb = []
                for fc in range(16):
                    t = pw.tile([128, 512], BF, tag=f"w2_{fc}")
                    nc.sync.dma_start(out=t, in_=w2t[l, fc * 128:(fc + 1) * 128, :])
                    w2_sb.append(t)
                b1_sb = load_bias_col("b1_sb", b1, l, 16)
                b2r_sb = load_row("b2r_sb", b2r, l)

                fT = []
                for fc in range(16):
                    pf = pps.tile([128, 512], F32, tag="mm")
                    for kt in range(4):
                        nc.tensor.matmul(pf, w1_sb[kt][:, fc * 128:(fc + 1) * 128],
                                         hT[kt], start=kt == 0, stop=kt == 3)
                    f_t = pes.tile([128, 512], BF, tag=f"fT_{fc}", bufs=1)
                    nc.scalar.activation(out=f_t, in_=pf,
                                         func=mybir.ActivationFunctionType.Relu,
                                         bias=b1_sb[:, fc:fc + 1])
                    fT.append(f_t)
                gb = gb_tiles("ffn", l)
                hbs = []
                for tt in range(4):
                    po = pps.tile([128, 512], F32, tag="mm")
                    for fc in range(16):
                        nc.tensor.matmul(po, fT[fc][:, tt * 128:(tt + 1) * 128],
                                         w2_sb[fc], start=fc == 0, stop=False)
                    nc.tensor.matmul(po, ones1, b2r_sb, start=False, stop=True)
                    hbs.append(residual_ln(tt, po, gb))
                retranspose(hbs)

            if debug:
                dbg = {n: dout(f"dbg_{n}", [LD, D], F32) for n in
                       ("h_sa", "h_ca", "h_ffn")}
                dbg_rbc = dout("dbg_rbc", [128, 512], F32)
                dbg_ctxT = dout("dbg_ctxT", [LD, D], BF)

            for l in range(nl):
                attention("sa", l, causal=True, kvT_src=hT, out_dram=saw)
                if debug and l == 0:
                    for tt in range(4):
                        nc.sync.dma_start(out=dbg["h_sa"][tt * 128:(tt + 1) * 128, :], in_=h[tt])
                attention("ca", l, causal=False, kvT_src=encT_sb, out_dram=caw)
                if debug and l == 0:
                    for tt in range(4):
                        nc.sync.dma_start(out=dbg["h_ca"][tt * 128:(tt + 1) * 128, :], in_=h[tt])
                ffn(l)
                if debug and l == 0:
                    for tt in range(4):
                        nc.sync.dma_start(out=dbg["h_ffn"][tt * 128:(tt + 1) * 128, :], in_=h[tt])

            for tt in range(4):
                nc.sync.dma_start(out=x_out[tt * 128:(tt + 1) * 128, :], in_=h[tt])

    nc.compile()
    _BUILD_CACHE[key] = nc
    return nc


# --------------------------------------------------------------------------
# host wrapper
# --------------------------------------------------------------------------

def kernel(**inputs):
    inp = {k: np.asarray(v) for k, v in inputs.items()}
    nl = NL_BUILD
    n_cores = N_CORES

    dec = inp["dec_inputs"].astype(np.int64)
    enc = inp["enc_inputs"].astype(np.int64)
    enc_out = inp["enc_outputs"].astype(np.float32)
    emb = inp["emb_table"].astype(np.float32)

    pos = sin_table(LD + 1, D)[1:LD + 1]
    x0_all = (emb[dec] + pos[None]).astype(np.float32)          # [B, LD, D]
    encT_all = np.ascontiguousarray(enc_out.transpose(0, 2, 1)).astype(BF16)

    affine = False
    for p in ("sa", "ca", "ffn"):
        if not (np.all(inp[f"{p}_g"] == 1.0) and np.all(inp[f"{p}_b"] == 0.0)):
            affine = True

    host_w = {}
    for p in ("sa", "ca"):
        host_w[f"{p}_wqt"] = np.ascontiguousarray(inp[f"{p}_wq"][:nl].transpose(0, 2, 1)).astype(BF16)
        host_w[f"{p}_wkt"] = np.ascontiguousarray(inp[f"{p}_wk"][:nl].transpose(0, 2, 1)).astype(BF16)
        host_w[f"{p}_wvt"] = np.ascontiguousarray(inp[f"{p}_wv"][:nl].transpose(0, 2, 1)).astype(BF16)
        host_w[f"{p}_wot"] = np.ascontiguousarray(inp[f"{p}_wo"][:nl].transpose(0, 2, 1)).astype(BF16)
        host_w[f"{p}_bqs"] = (inp[f"{p}_bq"][:nl] * SCALE).astype(np.float32)
        host_w[f"{p}_bk"] = inp[f"{p}_bk"][:nl].astype(np.float32)
        host_w[f"{p}_bv"] = inp[f"{p}_bv"][:nl].astype(np.float32)
        host_w[f"{p}_bo"] = inp[f"{p}_bo"][:nl].astype(np.float32).astype(BF16)
    host_w["w1t"] = np.ascontiguousarray(inp["ffn_w1"][:nl].transpose(0, 2, 1)).astype(BF16)
    host_w["w2t"] = np.ascontiguousarray(inp["ffn_w2"][:nl].transpose(0, 2, 1)).astype(BF16)
    host_w["b1"] = inp["ffn_b1"][:nl].astype(np.float32)
    host_w["b2r"] = inp["ffn_b2"][:nl].astype(np.float32).astype(BF16)
    if affine:
        for p in ("sa", "ca", "ffn"):
            host_w[f"{p}_g"] = inp[f"{p}_g"][:nl].astype(np.float32)
            host_w[f"{p}_b"] = inp[f"{p}_b"][:nl].astype(np.float32)

    tri = np.triu(np.ones((128, 128), np.float32), k=1) * NEG     # [q,k] block
    bias_qk = np.zeros((4, 128, 512), np.float32)
    for qc in range(4):
        bias_qk[qc, :, qc * 128:(qc + 1) * 128] += tri
    pad_dec_all = (dec == 0).astype(np.float32) * NEG             # [B, LD]
    pad_enc_all = (enc == 0).astype(np.float32) * NEG             # [B, LE]
    host_w["triT"] = np.ascontiguousarray(tri.T)
    host_w["ident_f"] = np.eye(128, dtype=np.float32)
    host_w["ident_b"] = np.eye(128, dtype=np.float32).astype(BF16)
    host_w["ones1"] = np.ones((1, 128), np.float32).astype(BF16)

    nc = build_kernel(affine, nl, n_cores)

    in_maps = []
    for b in range(n_cores):
        m = dict(host_w)
        m["x0"] = x0_all[b]
        m["x0T"] = np.ascontiguousarray(x0_all[b].T).astype(BF16)
        m["encT"] = encT_all[b]
        m["bias_qk"] = bias_qk + pad_dec_all[b][None, None, :]
        m["pad_dec"] = pad_dec_all[b]
        m["pad_enc"] = pad_enc_all[b]
        m["pad_enc_row"] = pad_enc_all[b][None, :].astype(BF16)
        in_maps.append(m)

    trace = os.environ.get("KERNEL_TRACE") == "1"
    res = bass_utils.run_bass_kernel_spmd(nc, in_maps, core_ids=list(range(n_cores)),
                                          trace=trace)
    global LAST_RESULT
    LAST_RESULT = res

    x = np.zeros((B, LD, D), np.float32)
    sa_ws = np.zeros((NL, B, H, LD, LD), np.float32)
    ca_ws = np.zeros((NL, B, H, LD, LE), np.float32)
    for b in range(n_cores):
        r = res.results[b]
        x[b] = r["x_out"]
        sa_ws[:nl, b] = r["saw"]
        ca_ws[:nl, b] = r["caw"]
    return x, sa_ws, ca_ws
